# revision 22
# baseline (speedup 1.0000x reference)
"""CTRNN (leaky-relu recurrence) Trainium2 Bass kernel.

Problem: out[t] = h_{t+1} = relu(0.8*h_t + 0.2*(x_t @ W_in.T + b_in + h_t @ W_hh.T + b_hh + noise_t))
Shapes: x [512, 256, 64], hidden [256, 256], noise_seq [512, 256],
        W_in [256, 64], W_hh [256, 256], b [256]. Returns (out [512,256,256], h_last [256,256]).

Strategy:
  - Data-parallel over batch: 8 cores x 32 batch each; weights replicated.
  - Host (numpy) precomputes layouts: W_aug = 0.8*I + 0.2*W_hh folded into one
    matrix so the device step is h' = relu(W_aug @ h + aW_in @ x_t + bias_t);
    everything transposed into [K(partitions), M] form, quantized to fp16
    (measured end-to-end rel err ~1.4e-3; fp32 path available via DTYPE env).
  - Device: pure 512-step scan. Per step, 2 H-chunks of 128; per chunk 3
    accumulating matmuls into one PSUM bank (K=64 x-proj + 2x K=128 recurrent),
    then one ScalarE Relu with per-partition bias (noise+biases) writing fp16
    h straight into an SBUF history buffer. History streams to DRAM in 1MB DMAs.
  - Host un-permutes the device layout into [T, B, H] fp32.
"""

import os

import numpy as np

T, B, I, H = 512, 256, 64, 256
N_CORES = 8
B_SH = B // N_CORES  # 32
ALPHA = 20.0 / 100.0
OM_ALPHA = 1.0 - ALPHA

# fp16 (default) or fp32 compute for matmul operands / h state
_DT = os.environ.get("CTRNN_DTYPE", "fp16")

_CACHE = {}
LAST_RESULTS = None  # BassKernelResults of the most recent run (for test harness)


def _build():
    import concourse.bass as bass
    import concourse.mybir as mybir
    import concourse.tile as tile
    from concourse import bacc

    fp = mybir.dt.float16 if _DT == "fp16" else mybir.dt.float32
    f32 = mybir.dt.float32

    nc = bacc.Bacc("TRN2", debug=False, num_devices=N_CORES)

    xT_d = nc.dram_tensor("xT", [I, T * B_SH], fp, kind="ExternalInput")
    wstk_d = nc.dram_tensor("wstk", [128, 4 * 128], fp, kind="ExternalInput")
    wx_d = nc.dram_tensor("wx", [I, 2 * 128], fp, kind="ExternalInput")
    nb_d = nc.dram_tensor("nb", [128, 2 * T], f32, kind="ExternalInput")
    h0_d = nc.dram_tensor("h0", [128, 64], fp, kind="ExternalInput")
    out_d = nc.dram_tensor("out_dev", [128, T * 64], fp, kind="ExternalOutput")

    with tile.TileContext(nc) as tc:
        with (
            tc.tile_pool(name="const", bufs=1) as cpool,
            tc.tile_pool(name="psum", bufs=4, space=bass.MemorySpace.PSUM) as pp,
        ):
            xT = cpool.tile([I, T * B_SH], fp)
            wstk = cpool.tile([128, 4 * 128], fp)
            wx = cpool.tile([I, 2 * 128], fp)
            nb = cpool.tile([128, 2 * T], f32)
            # h history: h_t at cols [t*64, (t+1)*64); col c*32+b = h[c*128+p, b]
            hbuf = cpool.tile([128, (T + 1) * 64], fp)

            nc.sync.dma_start(wstk[:], wstk_d[:])
            nc.sync.dma_start(wx[:], wx_d[:])
            nc.sync.dma_start(nb[:], nb_d[:])
            nc.sync.dma_start(hbuf[:, 0:64], h0_d[:])
            # chunked xT load: the scan's step 0 only needs the first slice,
            # so it starts ~1-2us in instead of waiting for the full 2MB
            XCH = T * B_SH // 8
            for g in range(8):
                nc.sync.dma_start(xT[:, g * XCH : (g + 1) * XCH], xT_d[:, g * XCH : (g + 1) * XCH])

            relu = mybir.ActivationFunctionType.Relu
            add_op = mybir.AluOpType.add
            max_op = mybir.AluOpType.max


            # HAM warm-up: the scan's PE duty cycle (~35%) is too low to
            # trigger the 2.4GHz un-throttle on its own, leaving the first
            # ~125us at K=4/8. Burn ~9us of dense matmuls (into a scratch
            # PSUM bank, result never read) while the xT DMA streams in.
            warm = pp.tile([128, 512], f32, name="warm", tag="ps0")
            with tc.tile_wait_until(0.0):
                for _ in range(8):
                    nc.tensor.matmul(warm[:], wstk[:, 0:128], wstk[:, 0:512], start=True, stop=True)

            # Manual schedule control: model-time floors (tile_wait_until)
            # dictate the scheduler's placement. x-projection MMs of step t
            # are floored half a step early so they execute in the PE-idle
            # window while step t-1's relus run, never gating the relu chain.
            SMS = 6e-4  # ~expected step period in ms
            # Floors must exceed the scheduler-model's DMA-prologue time or
            # they are ignored for early steps (order falls back to the
            # heuristic x-in-middle placement). They only control order, so
            # a generous offset costs nothing at runtime.
            OFF = 0.03
            for t in range(T):
                xin = xT[:, t * B_SH : (t + 1) * B_SH]
                htop = hbuf[:, t * 64 : t * 64 + 32]
                hbot = hbuf[:, t * 64 + 32 : t * 64 + 64]
                ps = [
                    pp.tile([128, B_SH], f32, name=f"ps{c}_{t}", tag=f"ps{c}")
                    for c in range(2)
                ]
                with tc.tile_wait_until(OFF + (t - 0.5) * SMS):
                    for c in range(2):
                        nc.tensor.matmul(ps[c][:], wx[:, c * 128 : (c + 1) * 128], xin, start=True, stop=False)
                with tc.tile_wait_until(OFF + t * SMS):
                    # ktops (gated by the early DVE relu), then kbots (gated
                    # by the late ACT relu)
                    for c in range(2):
                        nc.tensor.matmul(ps[c][:], wstk[:, c * 128 : (c + 1) * 128], htop, start=False, stop=False)
                    for c in (1, 0):
                        nc.tensor.matmul(ps[c][:], wstk[:, (2 + c) * 128 : (3 + c) * 128], hbot, start=False, stop=True)
                    # chunk 0 relu on DVE (fused add+max), chunk 1 on ACT
                    nc.vector.tensor_scalar(
                        hbuf[:, (t + 1) * 64 : (t + 1) * 64 + 32],
                        ps[0][:],
                        nb[:, t : t + 1],
                        0.0,
                        add_op,
                        max_op,
                    )
                    nc.scalar.activation(
                        hbuf[:, (t + 1) * 64 + 32 : (t + 1) * 64 + 64],
                        ps[1][:],
                        relu,
                        bias=nb[:, T + t : T + t + 1],
                        scale=1.0,
                    )

            CH = 64  # steps per output DMA -> [128, 4096] (1MB fp16)
            for s in range(0, T, CH):
                nc.sync.dma_start(
                    out_d[:, s * 64 : (s + CH) * 64],
                    hbuf[:, (s + 1) * 64 : (s + 1 + CH) * 64],
                )

    nc.compile()
    return nc


def _prep_shared(noise_seq, W_in, b_in, b_hh, W_hh):
    npdt = np.float16 if _DT == "fp16" else np.float32
    W_aug = (OM_ALPHA * np.eye(H, dtype=np.float32) + ALPHA * W_hh.astype(np.float32)).astype(npdt)
    WT = np.ascontiguousarray(W_aug.T)  # [H_in, H_out]
    wstk = np.concatenate(
        [WT[ci * 128 : (ci + 1) * 128, co * 128 : (co + 1) * 128] for ci in range(2) for co in range(2)],
        axis=1,
    )  # [128, 512], block order ci*2+co
    wx = np.ascontiguousarray((ALPHA * W_in.astype(np.float32)).T.astype(npdt))  # [64, 256]
    base = (ALPHA * (noise_seq.astype(np.float32) + b_in + b_hh)).astype(np.float32)  # [T, H]
    nbT = np.ascontiguousarray(base.T)  # [256, 512]
    nb = np.concatenate([nbT[0:128, :], nbT[128:256, :]], axis=1)  # [128, 1024]
    return np.ascontiguousarray(wstk), wx, np.ascontiguousarray(nb)


def kernel(x, hidden, noise_seq, W_in, b_in, W_hh, b_hh, _trace=False):
    global LAST_RESULTS
    from concourse.bass_utils import run_bass_kernel_spmd

    x = np.asarray(x, dtype=np.float32)
    hidden = np.asarray(hidden, dtype=np.float32)
    noise_seq = np.asarray(noise_seq, dtype=np.float32)
    W_in = np.asarray(W_in, dtype=np.float32)
    b_in = np.asarray(b_in, dtype=np.float32)
    W_hh = np.asarray(W_hh, dtype=np.float32)
    b_hh = np.asarray(b_hh, dtype=np.float32)

    if "nc" not in _CACHE:
        _CACHE["nc"] = _build()
    nc = _CACHE["nc"]

    npdt = np.float16 if _DT == "fp16" else np.float32
    wstk, wx, nb = _prep_shared(noise_seq, W_in, b_in, b_hh, W_hh)

    in_maps = []
    for core in range(N_CORES):
        b0 = core * B_SH
        xc = x[:, b0 : b0 + B_SH, :]  # [T, 32, 64]
        xT = np.ascontiguousarray(xc.transpose(2, 0, 1).reshape(I, T * B_SH)).astype(npdt)
        hc = hidden[b0 : b0 + B_SH, :]  # [32, 256]
        h0 = np.ascontiguousarray(
            hc.T.reshape(2, 128, B_SH).transpose(1, 0, 2).reshape(128, 64)
        ).astype(npdt)
        in_maps.append({"xT": xT, "wstk": wstk, "wx": wx, "nb": nb, "h0": h0})

    res = run_bass_kernel_spmd(nc, in_maps, core_ids=list(range(N_CORES)), trace=_trace)
    LAST_RESULTS = res

    out = np.empty((T, B, H), dtype=np.float32)
    for core in range(N_CORES):
        od = res.results[core]["out_dev"]  # [128, T*64]
        o = od.reshape(128, T, 2, B_SH).transpose(1, 3, 2, 0).reshape(T, B_SH, H)
        out[:, core * B_SH : (core + 1) * B_SH, :] = o.astype(np.float32)
    h_last = out[-1].copy()
    return out, h_last


# revision 23
# speedup vs baseline: 1.0550x; 1.0550x over previous
"""CTRNN (leaky-relu recurrence) Trainium2 Bass kernel.

Problem: out[t] = h_{t+1} = relu(0.8*h_t + 0.2*(x_t @ W_in.T + b_in + h_t @ W_hh.T + b_hh + noise_t))
Shapes: x [512, 256, 64], hidden [256, 256], noise_seq [512, 256],
        W_in [256, 64], W_hh [256, 256], b [256]. Returns (out [512,256,256], h_last [256,256]).

Strategy:
  - Data-parallel over batch: 8 cores x 32 batch each; weights replicated.
  - Host (numpy) precomputes layouts: W_aug = 0.8*I + 0.2*W_hh folded into one
    matrix so the device step is h' = relu(W_aug @ h + aW_in @ x_t + bias_t);
    everything transposed into [K(partitions), M] form, quantized to fp16
    (measured end-to-end rel err ~1.4e-3; fp32 path available via DTYPE env).
  - Device: pure 512-step scan. Per step, 2 H-chunks of 128; per chunk 3
    accumulating matmuls into one PSUM bank (K=64 x-proj + 2x K=128 recurrent),
    then one ScalarE Relu with per-partition bias (noise+biases) writing fp16
    h straight into an SBUF history buffer. History streams to DRAM in 1MB DMAs.
  - Host un-permutes the device layout into [T, B, H] fp32.
"""

import os

import numpy as np

T, B, I, H = 512, 256, 64, 256
N_CORES = 8
B_SH = B // N_CORES  # 32
ALPHA = 20.0 / 100.0
OM_ALPHA = 1.0 - ALPHA

# fp16 (default) or fp32 compute for matmul operands / h state
_DT = os.environ.get("CTRNN_DTYPE", "fp16")

_CACHE = {}
LAST_RESULTS = None  # BassKernelResults of the most recent run (for test harness)


def _build():
    import concourse.bass as bass
    import concourse.mybir as mybir
    import concourse.tile as tile
    from concourse import bacc

    fp = mybir.dt.float16 if _DT == "fp16" else mybir.dt.float32
    f32 = mybir.dt.float32

    nc = bacc.Bacc("TRN2", debug=False, num_devices=N_CORES)

    xT_d = nc.dram_tensor("xT", [I, T * B_SH], fp, kind="ExternalInput")
    wstk_d = nc.dram_tensor("wstk", [128, 4 * 128], fp, kind="ExternalInput")
    wx_d = nc.dram_tensor("wx", [I, 2 * 128], fp, kind="ExternalInput")
    nb_d = nc.dram_tensor("nb", [128, 2 * T], f32, kind="ExternalInput")
    h0_d = nc.dram_tensor("h0", [128, 64], fp, kind="ExternalInput")
    out_d = nc.dram_tensor("out_dev", [128, T * 64], fp, kind="ExternalOutput")

    with tile.TileContext(nc) as tc:
        with (
            tc.tile_pool(name="const", bufs=1) as cpool,
            tc.tile_pool(name="psum", bufs=4, space=bass.MemorySpace.PSUM) as pp,
        ):
            xT = cpool.tile([I, T * B_SH], fp)
            wstk = cpool.tile([128, 4 * 128], fp)
            wx = cpool.tile([I, 2 * 128], fp)
            nb = cpool.tile([128, 2 * T], f32)
            # h history: h_t at cols [t*64, (t+1)*64); col c*32+b = h[c*128+p, b]
            hbuf = cpool.tile([128, (T + 1) * 64], fp)

            nc.sync.dma_start(wstk[:], wstk_d[:])
            nc.sync.dma_start(wx[:], wx_d[:])
            nc.sync.dma_start(nb[:], nb_d[:])
            nc.sync.dma_start(hbuf[:, 0:64], h0_d[:])
            # chunked xT load: the scan's step 0 only needs the first slice,
            # so it starts ~1-2us in instead of waiting for the full 2MB
            XCH = T * B_SH // 8
            for g in range(8):
                nc.sync.dma_start(xT[:, g * XCH : (g + 1) * XCH], xT_d[:, g * XCH : (g + 1) * XCH])

            relu = mybir.ActivationFunctionType.Relu
            add_op = mybir.AluOpType.add
            max_op = mybir.AluOpType.max


            # HAM warm-up: the scan's PE duty cycle (~35%) is too low to
            # trigger the 2.4GHz un-throttle on its own, leaving the first
            # ~125us at K=4/8. Burn ~9us of dense matmuls (into a scratch
            # PSUM bank, result never read) while the xT DMA streams in.
            warm = pp.tile([128, 512], f32, name="warm", tag="ps0")
            with tc.tile_wait_until(0.0):
                for _ in range(5):
                    nc.tensor.matmul(warm[:], wstk[:, 0:128], wstk[:, 0:512], start=True, stop=True)

            # Manual schedule control: model-time floors (tile_wait_until)
            # dictate the scheduler's placement. x-projection MMs of step t
            # are floored half a step early so they execute in the PE-idle
            # window while step t-1's relus run, never gating the relu chain.
            SMS = 6e-4  # ~expected step period in ms
            # Floors must exceed the scheduler-model's DMA-prologue time or
            # they are ignored for early steps (order falls back to the
            # heuristic x-in-middle placement). They only control order, so
            # a generous offset costs nothing at runtime.
            OFF = 0.03
            for t in range(T):
                xin = xT[:, t * B_SH : (t + 1) * B_SH]
                htop = hbuf[:, t * 64 : t * 64 + 32]
                hbot = hbuf[:, t * 64 + 32 : t * 64 + 64]
                ps = [
                    pp.tile([128, B_SH], f32, name=f"ps{c}_{t}", tag=f"ps{c}")
                    for c in range(2)
                ]
                with tc.tile_wait_until(OFF + (t - 0.5) * SMS):
                    for c in range(2):
                        nc.tensor.matmul(ps[c][:], wx[:, c * 128 : (c + 1) * 128], xin, start=True, stop=False)
                with tc.tile_wait_until(OFF + t * SMS):
                    # ktops (gated by the early DVE relu), then kbots (gated
                    # by the late ACT relu)
                    for c in range(2):
                        nc.tensor.matmul(ps[c][:], wstk[:, c * 128 : (c + 1) * 128], htop, start=False, stop=False)
                    for c in range(2):
                        nc.tensor.matmul(ps[c][:], wstk[:, (2 + c) * 128 : (3 + c) * 128], hbot, start=False, stop=True)
                    # chunk 0 relu on DVE (fused add+max), chunk 1 on ACT
                    nc.vector.tensor_scalar(
                        hbuf[:, (t + 1) * 64 : (t + 1) * 64 + 32],
                        ps[0][:],
                        nb[:, t : t + 1],
                        0.0,
                        add_op,
                        max_op,
                    )
                    nc.scalar.activation(
                        hbuf[:, (t + 1) * 64 + 32 : (t + 1) * 64 + 64],
                        ps[1][:],
                        relu,
                        bias=nb[:, T + t : T + t + 1],
                        scale=1.0,
                    )

            CH = 64  # steps per output DMA -> [128, 4096] (1MB fp16)
            for s in range(0, T - CH, CH):
                nc.sync.dma_start(
                    out_d[:, s * 64 : (s + CH) * 64],
                    hbuf[:, (s + 1) * 64 : (s + 1 + CH) * 64],
                )
            # tail region in 16-step pieces: the last DMA (after step 511)
            # moves only 256KB instead of 1MB
            for s in range(T - CH, T, 16):
                nc.sync.dma_start(
                    out_d[:, s * 64 : (s + 16) * 64],
                    hbuf[:, (s + 1) * 64 : (s + 1 + 16) * 64],
                )

    nc.compile()
    return nc


def _prep_shared(noise_seq, W_in, b_in, b_hh, W_hh):
    npdt = np.float16 if _DT == "fp16" else np.float32
    W_aug = (OM_ALPHA * np.eye(H, dtype=np.float32) + ALPHA * W_hh.astype(np.float32)).astype(npdt)
    WT = np.ascontiguousarray(W_aug.T)  # [H_in, H_out]
    wstk = np.concatenate(
        [WT[ci * 128 : (ci + 1) * 128, co * 128 : (co + 1) * 128] for ci in range(2) for co in range(2)],
        axis=1,
    )  # [128, 512], block order ci*2+co
    wx = np.ascontiguousarray((ALPHA * W_in.astype(np.float32)).T.astype(npdt))  # [64, 256]
    base = (ALPHA * (noise_seq.astype(np.float32) + b_in + b_hh)).astype(np.float32)  # [T, H]
    nbT = np.ascontiguousarray(base.T)  # [256, 512]
    nb = np.concatenate([nbT[0:128, :], nbT[128:256, :]], axis=1)  # [128, 1024]
    return np.ascontiguousarray(wstk), wx, np.ascontiguousarray(nb)


def kernel(x, hidden, noise_seq, W_in, b_in, W_hh, b_hh, _trace=False):
    global LAST_RESULTS
    from concourse.bass_utils import run_bass_kernel_spmd

    x = np.asarray(x, dtype=np.float32)
    hidden = np.asarray(hidden, dtype=np.float32)
    noise_seq = np.asarray(noise_seq, dtype=np.float32)
    W_in = np.asarray(W_in, dtype=np.float32)
    b_in = np.asarray(b_in, dtype=np.float32)
    W_hh = np.asarray(W_hh, dtype=np.float32)
    b_hh = np.asarray(b_hh, dtype=np.float32)

    if "nc" not in _CACHE:
        _CACHE["nc"] = _build()
    nc = _CACHE["nc"]

    npdt = np.float16 if _DT == "fp16" else np.float32
    wstk, wx, nb = _prep_shared(noise_seq, W_in, b_in, b_hh, W_hh)

    in_maps = []
    for core in range(N_CORES):
        b0 = core * B_SH
        xc = x[:, b0 : b0 + B_SH, :]  # [T, 32, 64]
        xT = np.ascontiguousarray(xc.transpose(2, 0, 1).reshape(I, T * B_SH)).astype(npdt)
        hc = hidden[b0 : b0 + B_SH, :]  # [32, 256]
        h0 = np.ascontiguousarray(
            hc.T.reshape(2, 128, B_SH).transpose(1, 0, 2).reshape(128, 64)
        ).astype(npdt)
        in_maps.append({"xT": xT, "wstk": wstk, "wx": wx, "nb": nb, "h0": h0})

    res = run_bass_kernel_spmd(nc, in_maps, core_ids=list(range(N_CORES)), trace=_trace)
    LAST_RESULTS = res

    out = np.empty((T, B, H), dtype=np.float32)
    for core in range(N_CORES):
        od = res.results[core]["out_dev"]  # [128, T*64]
        o = od.reshape(128, T, 2, B_SH).transpose(1, 3, 2, 0).reshape(T, B_SH, H)
        out[:, core * B_SH : (core + 1) * B_SH, :] = o.astype(np.float32)
    h_last = out[-1].copy()
    return out, h_last


# revision 24
# speedup vs baseline: 1.0589x; 1.0037x over previous
"""CTRNN (leaky-relu recurrence) Trainium2 Bass kernel.

Problem: out[t] = h_{t+1} = relu(0.8*h_t + 0.2*(x_t @ W_in.T + b_in + h_t @ W_hh.T + b_hh + noise_t))
Shapes: x [512, 256, 64], hidden [256, 256], noise_seq [512, 256],
        W_in [256, 64], W_hh [256, 256], b [256]. Returns (out [512,256,256], h_last [256,256]).

Strategy:
  - Data-parallel over batch: 8 cores x 32 batch each; weights replicated.
  - Host (numpy) precomputes layouts: W_aug = 0.8*I + 0.2*W_hh folded into one
    matrix so the device step is h' = relu(W_aug @ h + aW_in @ x_t + bias_t);
    everything transposed into [K(partitions), M] form, quantized to fp16
    (measured end-to-end rel err ~1.4e-3; fp32 path available via DTYPE env).
  - Device: pure 512-step scan. Per step, 2 H-chunks of 128; per chunk 3
    accumulating matmuls into one PSUM bank (K=64 x-proj + 2x K=128 recurrent),
    then one ScalarE Relu with per-partition bias (noise+biases) writing fp16
    h straight into an SBUF history buffer. History streams to DRAM in 1MB DMAs.
  - Host un-permutes the device layout into [T, B, H] fp32.
"""

import os

import numpy as np

T, B, I, H = 512, 256, 64, 256
N_CORES = 8
B_SH = B // N_CORES  # 32
ALPHA = 20.0 / 100.0
OM_ALPHA = 1.0 - ALPHA

# fp16 (default) or fp32 compute for matmul operands / h state
_DT = os.environ.get("CTRNN_DTYPE", "fp16")

_CACHE = {}
LAST_RESULTS = None  # BassKernelResults of the most recent run (for test harness)


def _build():
    import concourse.bass as bass
    import concourse.mybir as mybir
    import concourse.tile as tile
    from concourse import bacc

    fp = mybir.dt.float16 if _DT == "fp16" else mybir.dt.float32
    f32 = mybir.dt.float32

    nc = bacc.Bacc("TRN2", debug=False, num_devices=N_CORES)

    xT_d = nc.dram_tensor("xT", [I, T * B_SH], fp, kind="ExternalInput")
    wstk_d = nc.dram_tensor("wstk", [128, 4 * 128], fp, kind="ExternalInput")
    wx_d = nc.dram_tensor("wx", [I, 2 * 128], fp, kind="ExternalInput")
    nb_d = nc.dram_tensor("nb", [128, 2 * T], f32, kind="ExternalInput")
    h0_d = nc.dram_tensor("h0", [128, 64], fp, kind="ExternalInput")
    out_d = nc.dram_tensor("out_dev", [128, T * 64], fp, kind="ExternalOutput")

    with tile.TileContext(nc) as tc:
        with (
            tc.tile_pool(name="const", bufs=1) as cpool,
            tc.tile_pool(name="psum", bufs=4, space=bass.MemorySpace.PSUM) as pp,
        ):
            xT = cpool.tile([I, T * B_SH], fp)
            wstk = cpool.tile([128, 4 * 128], fp)
            wx = cpool.tile([I, 2 * 128], fp)
            nb = cpool.tile([128, 2 * T], f32)
            # h history: h_t at cols [t*64, (t+1)*64); col c*32+b = h[c*128+p, b]
            hbuf = cpool.tile([128, (T + 1) * 64], fp)

            nc.sync.dma_start(wstk[:], wstk_d[:])
            nc.sync.dma_start(wx[:], wx_d[:])
            nc.sync.dma_start(nb[:], nb_d[:])
            nc.sync.dma_start(hbuf[:, 0:64], h0_d[:])
            # chunked xT load: the scan's step 0 only needs the first slice,
            # so it starts ~1-2us in instead of waiting for the full 2MB
            XCH = T * B_SH // 8
            for g in range(8):
                nc.sync.dma_start(xT[:, g * XCH : (g + 1) * XCH], xT_d[:, g * XCH : (g + 1) * XCH])

            relu = mybir.ActivationFunctionType.Relu
            add_op = mybir.AluOpType.add
            max_op = mybir.AluOpType.max


            # HAM warm-up: the scan's PE duty cycle (~35%) is too low to
            # trigger the 2.4GHz un-throttle on its own, leaving the first
            # ~125us at K=4/8. Burn ~9us of dense matmuls (into a scratch
            # PSUM bank, result never read) while the xT DMA streams in.
            warm = pp.tile([128, 512], f32, name="warm", tag="ps0")
            act_scratch = cpool.tile([128, 1], f32, name="act_scratch")
            with tc.tile_wait_until(0.0):
                for _ in range(5):
                    nc.tensor.matmul(warm[:], wstk[:, 0:128], wstk[:, 0:512], start=True, stop=True)
                # dummy ACTIVATE: pulls the ~2.7us Relu table load off step 0's
                # critical chain (runs while the xT DMA streams in)
                nc.scalar.activation(act_scratch[:], wstk[:, 0:1], relu)

            # Manual schedule control: model-time floors (tile_wait_until)
            # dictate the scheduler's placement. x-projection MMs of step t
            # are floored half a step early so they execute in the PE-idle
            # window while step t-1's relus run, never gating the relu chain.
            SMS = 6e-4  # ~expected step period in ms
            # Floors must exceed the scheduler-model's DMA-prologue time or
            # they are ignored for early steps (order falls back to the
            # heuristic x-in-middle placement). They only control order, so
            # a generous offset costs nothing at runtime.
            OFF = 0.03
            for t in range(T):
                xin = xT[:, t * B_SH : (t + 1) * B_SH]
                htop = hbuf[:, t * 64 : t * 64 + 32]
                hbot = hbuf[:, t * 64 + 32 : t * 64 + 64]
                ps = [
                    pp.tile([128, B_SH], f32, name=f"ps{c}_{t}", tag=f"ps{c}")
                    for c in range(2)
                ]
                with tc.tile_wait_until(OFF + (t - 0.5) * SMS):
                    for c in range(2):
                        nc.tensor.matmul(ps[c][:], wx[:, c * 128 : (c + 1) * 128], xin, start=True, stop=False)
                with tc.tile_wait_until(OFF + t * SMS):
                    # ktops (gated by the early DVE relu), then kbots (gated
                    # by the late ACT relu)
                    for c in range(2):
                        nc.tensor.matmul(ps[c][:], wstk[:, c * 128 : (c + 1) * 128], htop, start=False, stop=False)
                    for c in range(2):
                        nc.tensor.matmul(ps[c][:], wstk[:, (2 + c) * 128 : (3 + c) * 128], hbot, start=False, stop=True)
                    # chunk 0 relu on DVE (fused add+max), chunk 1 on ACT
                    nc.vector.tensor_scalar(
                        hbuf[:, (t + 1) * 64 : (t + 1) * 64 + 32],
                        ps[0][:],
                        nb[:, t : t + 1],
                        0.0,
                        add_op,
                        max_op,
                    )
                    nc.scalar.activation(
                        hbuf[:, (t + 1) * 64 + 32 : (t + 1) * 64 + 64],
                        ps[1][:],
                        relu,
                        bias=nb[:, T + t : T + t + 1],
                        scale=1.0,
                    )

            CH = 64  # steps per output DMA -> [128, 4096] (1MB fp16)
            for s in range(0, T - CH, CH):
                nc.sync.dma_start(
                    out_d[:, s * 64 : (s + CH) * 64],
                    hbuf[:, (s + 1) * 64 : (s + 1 + CH) * 64],
                )
            # tail region in 16-step pieces: the last DMA (after step 511)
            # moves only 256KB instead of 1MB
            for s in range(T - CH, T, 16):
                nc.sync.dma_start(
                    out_d[:, s * 64 : (s + 16) * 64],
                    hbuf[:, (s + 1) * 64 : (s + 1 + 16) * 64],
                )

    nc.compile()
    return nc


def _prep_shared(noise_seq, W_in, b_in, b_hh, W_hh):
    npdt = np.float16 if _DT == "fp16" else np.float32
    W_aug = (OM_ALPHA * np.eye(H, dtype=np.float32) + ALPHA * W_hh.astype(np.float32)).astype(npdt)
    WT = np.ascontiguousarray(W_aug.T)  # [H_in, H_out]
    wstk = np.concatenate(
        [WT[ci * 128 : (ci + 1) * 128, co * 128 : (co + 1) * 128] for ci in range(2) for co in range(2)],
        axis=1,
    )  # [128, 512], block order ci*2+co
    wx = np.ascontiguousarray((ALPHA * W_in.astype(np.float32)).T.astype(npdt))  # [64, 256]
    base = (ALPHA * (noise_seq.astype(np.float32) + b_in + b_hh)).astype(np.float32)  # [T, H]
    nbT = np.ascontiguousarray(base.T)  # [256, 512]
    nb = np.concatenate([nbT[0:128, :], nbT[128:256, :]], axis=1)  # [128, 1024]
    return np.ascontiguousarray(wstk), wx, np.ascontiguousarray(nb)


def kernel(x, hidden, noise_seq, W_in, b_in, W_hh, b_hh, _trace=False):
    global LAST_RESULTS
    from concourse.bass_utils import run_bass_kernel_spmd

    x = np.asarray(x, dtype=np.float32)
    hidden = np.asarray(hidden, dtype=np.float32)
    noise_seq = np.asarray(noise_seq, dtype=np.float32)
    W_in = np.asarray(W_in, dtype=np.float32)
    b_in = np.asarray(b_in, dtype=np.float32)
    W_hh = np.asarray(W_hh, dtype=np.float32)
    b_hh = np.asarray(b_hh, dtype=np.float32)

    if "nc" not in _CACHE:
        _CACHE["nc"] = _build()
    nc = _CACHE["nc"]

    npdt = np.float16 if _DT == "fp16" else np.float32
    wstk, wx, nb = _prep_shared(noise_seq, W_in, b_in, b_hh, W_hh)

    in_maps = []
    for core in range(N_CORES):
        b0 = core * B_SH
        xc = x[:, b0 : b0 + B_SH, :]  # [T, 32, 64]
        xT = np.ascontiguousarray(xc.transpose(2, 0, 1).reshape(I, T * B_SH)).astype(npdt)
        hc = hidden[b0 : b0 + B_SH, :]  # [32, 256]
        h0 = np.ascontiguousarray(
            hc.T.reshape(2, 128, B_SH).transpose(1, 0, 2).reshape(128, 64)
        ).astype(npdt)
        in_maps.append({"xT": xT, "wstk": wstk, "wx": wx, "nb": nb, "h0": h0})

    res = run_bass_kernel_spmd(nc, in_maps, core_ids=list(range(N_CORES)), trace=_trace)
    LAST_RESULTS = res

    out = np.empty((T, B, H), dtype=np.float32)
    for core in range(N_CORES):
        od = res.results[core]["out_dev"]  # [128, T*64]
        o = od.reshape(128, T, 2, B_SH).transpose(1, 3, 2, 0).reshape(T, B_SH, H)
        out[:, core * B_SH : (core + 1) * B_SH, :] = o.astype(np.float32)
    h_last = out[-1].copy()
    return out, h_last
